# revision 9
# baseline (speedup 1.0000x reference)
"""Multi-head attention (B=2,S=2048,E=1024,H=16) on 8 TRN2 NeuronCores.

Tensor-parallel over heads: core i owns heads 2i,2i+1 (128 output features).
Each core: full x [4096,1024], weight shards [128,1024], bias shards [128].
Attention computed in "transposed scores" orientation:
  qT/kT [d,s] layouts, scores^T [t,s] = kT_tile.T @ qT, exp on ACT,
  PV with V_ext=[V|ones] (denominator for free), PE transpose-back, scale.
"""

import sys

sys.path.insert(0, "/opt/trn_rl_repo")

import numpy as np

import concourse.bass as bass
import concourse.mybir as mybir
from concourse import bacc
from concourse.bass_utils import run_bass_kernel_spmd
from concourse.masks import make_identity
from concourse.tile import TileContext

B, S, E, H = 2, 2048, 1024, 16
D = E // H  # 64
NCORES = 8
HLOC = H // NCORES  # 2 heads per core
EH = HLOC * D  # 128 local output features
BS = B * S  # 4096
F16 = mybir.dt.float16
F32 = mybir.dt.float32
EXP = mybir.ActivationFunctionType.Exp

def _bcast_free(ap2d, n):
    """[P,1] AP -> [P,n] AP with free stride 0 (broadcast along free dim)."""
    return bass.AP(tensor=ap2d.tensor, offset=ap2d.offset, ap=[ap2d.ap[0], [0, n]])


NE = E // 128  # 8 e-tiles
NT = S // 128  # 16 t-tiles per batch
SBLK = 512  # s-block width for scores/PV
NSBLK = S // SBLK  # 4 per batch


def build():
    nc = bacc.Bacc(None, target_bir_lowering=False)
    x_d = nc.dram_tensor("x", [BS, E], F16, kind="ExternalInput")
    wq_d = nc.dram_tensor("wq", [EH, E], F16, kind="ExternalInput")
    wk_d = nc.dram_tensor("wk", [EH, E], F16, kind="ExternalInput")
    wv_d = nc.dram_tensor("wv", [EH, E], F16, kind="ExternalInput")
    bq_d = nc.dram_tensor("bq", [EH], F16, kind="ExternalInput")
    bk_d = nc.dram_tensor("bk", [EH], F16, kind="ExternalInput")
    bv_d = nc.dram_tensor("bv", [EH], F16, kind="ExternalInput")
    out_d = nc.dram_tensor("out", [BS, EH], F16, kind="ExternalOutput")

    with TileContext(nc) as tc:
        with (
            tc.tile_pool(name="singles", bufs=1) as singles,
            tc.tile_pool(name="pt_pool", bufs=6) as pt_pool,
            tc.tile_pool(name="ot_pool", bufs=3) as ot_pool,
            tc.tile_pool(name="ob_pool", bufs=8) as ob_pool,
            tc.tile_pool(name="rc_pool", bufs=8) as rc_pool,
            tc.tile_pool(name="big_ps", bufs=2, space="PSUM") as big_ps,
            tc.tile_pool(name="pv_ps", bufs=2, space="PSUM") as pv_ps,
            tc.tile_pool(name="aux_ps", bufs=2, space="PSUM") as aux_ps,
        ):
            # ---- constants ----
            identity = singles.tile([128, 128], F16, tag="identity")
            make_identity(nc, identity)
            bq_st = singles.tile([EH, 1], F16, tag="bq_st")
            bk_st = singles.tile([EH, 1], F16, tag="bk_st")
            nc.sync.dma_start(out=bq_st, in_=bq_d[:].rearrange("(p o) -> p o", o=1))
            nc.sync.dma_start(out=bk_st, in_=bk_d[:].rearrange("(p o) -> p o", o=1))
            bq_sb = singles.tile([EH, 1], F32, tag="bq_sb")
            bk_sb = singles.tile([EH, 1], F32, tag="bk_sb")
            nc.vector.tensor_copy(bq_sb, bq_st)
            nc.vector.tensor_copy(bk_sb, bk_st)
            # bv broadcast across partitions: [128 parts, 128 (=d) free]
            bv_bc = singles.tile([128, EH], F16, tag="bv_bc")
            _bv = bv_d[:]
            bv_ap = bass.AP(tensor=_bv.tensor, offset=_bv.offset, ap=[[0, 128], [1, EH]])
            nc.gpsimd.dma_start(out=bv_bc, in_=bv_ap)

            # ---- transposed weights: wT[k] = W[:, k*128:(k+1)*128].T ----
            wqT = singles.tile([128, NE, EH], F16, tag="wqT")
            wkT = singles.tile([128, NE, EH], F16, tag="wkT")
            wvT = singles.tile([128, NE, EH], F16, tag="wvT")
            for w_d, wT in ((wq_d, wqT), (wk_d, wkT), (wv_d, wvT)):
                for k in range(NE):
                    nc.sync.dma_start(
                        out=wT[:, k, :],
                        in_=w_d[:, k * 128 : (k + 1) * 128],
                        transpose=True,
                    )

            # ---- xT: [e,s] layout, 8 tiles of [128, 4096] ----
            xT = singles.tile([128, NE, BS], F16, tag="xT")
            XCH = 1024  # transpose-DMA chunk along s
            for k in range(NE):
                for c in range(BS // XCH):
                    nc.sync.dma_start(
                        out=xT[:, k, c * XCH : (c + 1) * XCH],
                        in_=x_d[c * XCH : (c + 1) * XCH, k * 128 : (k + 1) * 128],
                        transpose=True,
                    )

            # ---- Q/K projections -> qT/kT [128(d), 4096(s)] fp16 ----
            qT = singles.tile([128, BS], F16, tag="qT")
            kT = singles.tile([128, BS], F16, tag="kT")
            for wT, bias_sb, dstT in ((wqT, bq_sb, qT), (wkT, bk_sb, kT)):
                for blk in range(BS // SBLK):
                    sl = slice(blk * SBLK, (blk + 1) * SBLK)
                    ps = big_ps.tile([128, 2, SBLK], F32, tag="big")
                    for k in range(NE):
                        nc.tensor.matmul(
                            ps[:, 0, :],
                            lhsT=wT[:, k, :],
                            rhs=xT[:, k, sl],
                            start=(k == 0),
                            stop=(k == NE - 1),
                        )
                    nc.vector.tensor_add(
                        dstT[:, sl], ps[:, 0, :], _bcast_free(bias_sb, SBLK)
                    )

            # ---- V projection (direct [s,d] layout) + ones column ----
            # V_sb free layout per s-tile: [h*65 + (0..63)] = head h dims,
            # [h*65+64] = ones.
            NST = BS // 128  # 32 s-tiles
            v_sb = singles.tile([128, NST, 2 * (D + 1)], F16, tag="v_sb")
            nc.vector.memset(v_sb[:, :, D], 1.0)
            nc.vector.memset(v_sb[:, :, D + 1 + D], 1.0)
            for st in range(NST):
                ps = aux_ps.tile([128, EH], F32, tag="aux")
                for k in range(NE):
                    nc.tensor.matmul(
                        ps,
                        lhsT=xT[:, k, st * 128 : (st + 1) * 128],
                        rhs=wvT[:, k, :],
                        start=(k == 0),
                        stop=(k == NE - 1),
                    )
                for h in range(HLOC):
                    nc.vector.tensor_add(
                        v_sb[:, st, h * (D + 1) : h * (D + 1) + D],
                        ps[:, h * D : (h + 1) * D],
                        bv_bc[:, h * D : (h + 1) * D],
                    )

            # ---- attention per (b, h) ----
            for b in range(B):
                for h in range(HLOC):
                    hsl = slice(h * D, (h + 1) * D)
                    base = b * S
                    for sb in range(NSBLK):
                        ssl = slice(base + sb * SBLK, base + (sb + 1) * SBLK)
                        pv = pv_ps.tile([D + 1, SBLK], F32, tag="pv")
                        for tg in range(NT // 2):
                            sc = big_ps.tile([128, 2, SBLK], F32, tag="big")
                            for j in range(2):
                                t = tg * 2 + j
                                nc.tensor.matmul(
                                    sc[:, j, :],
                                    lhsT=kT[hsl, base + t * 128 : base + (t + 1) * 128],
                                    rhs=qT[hsl, ssl],
                                    start=True,
                                    stop=True,
                                    skip_group_check=True,
                                )
                            pt = pt_pool.tile([128, 2, SBLK], F16, tag="pt")
                            nc.scalar.activation(pt, sc, EXP, scale=float(1.0 / np.sqrt(D)))
                            for j in range(2):
                                t = tg * 2 + j
                                nc.tensor.matmul(
                                    pv,
                                    lhsT=v_sb[:, b * NT + t, h * (D + 1) : (h + 1) * (D + 1)],
                                    rhs=pt[:, j, :],
                                    start=(t == 0),
                                    stop=(t == NT - 1),
                                    skip_group_check=True,
                                )
                        # epilogue: copy out^T+denom to SBUF, transpose back,
                        # normalize, store.
                        ot = ot_pool.tile([D + 1, SBLK], F16, tag="ot")
                        nc.vector.tensor_copy(ot, pv)
                        for c in range(SBLK // 128):
                            tr = aux_ps.tile([128, D + 1], F16, tag="aux")
                            nc.tensor.matmul(
                                tr,
                                lhsT=ot[:, c * 128 : (c + 1) * 128],
                                rhs=identity[: D + 1, : D + 1],
                                is_transpose=True,
                                skip_group_check=True,
                            )
                            rc = rc_pool.tile([128, 1], F32, tag="rc")
                            nc.vector.reciprocal(rc, tr[:, D : D + 1])
                            ob = ob_pool.tile([128, D], F16, tag="ob")
                            nc.vector.tensor_mul(ob, tr[:, :D], _bcast_free(rc, D))
                            r0 = base + sb * SBLK + c * 128
                            nc.sync.dma_start(
                                out=out_d[r0 : r0 + 128, hsl], in_=ob
                            )
    return nc


_CACHE: dict = {}


def _get_nc():
    if "nc" not in _CACHE:
        nc = build()
        nc.finalize()
        _CACHE["nc"] = nc
    return _CACHE["nc"]


def _in_maps(inputs):
    x = np.ascontiguousarray(np.asarray(inputs["x"], dtype=np.float16).reshape(BS, E))
    maps = []
    for i in range(NCORES):
        sl = slice(i * EH, (i + 1) * EH)
        maps.append(
            {
                "x": x,
                "wq": np.ascontiguousarray(np.asarray(inputs["Wq"], np.float16)[sl]),
                "wk": np.ascontiguousarray(np.asarray(inputs["Wk"], np.float16)[sl]),
                "wv": np.ascontiguousarray(np.asarray(inputs["Wv"], np.float16)[sl]),
                "bq": np.ascontiguousarray(np.asarray(inputs["bq"], np.float16)[sl]),
                "bk": np.ascontiguousarray(np.asarray(inputs["bk"], np.float16)[sl]),
                "bv": np.ascontiguousarray(np.asarray(inputs["bv"], np.float16)[sl]),
            }
        )
    return maps


def _gather(results):
    out = np.concatenate([results[i]["out"] for i in range(NCORES)], axis=1)
    return out.reshape(B, S, E).astype(np.float16)


def kernel(**inputs):
    nc = _get_nc()
    res = run_bass_kernel_spmd(nc, _in_maps(inputs), list(range(NCORES)))
    return _gather(res.results)


def bench(inputs, trace=True, tmpdir=None):
    """Run with neuron-profile tracing; returns (out, exec_time_ns, results)."""
    nc = _get_nc()
    res = run_bass_kernel_spmd(
        nc, _in_maps(inputs), list(range(NCORES)), trace=trace, tmpdir=tmpdir
    )
    return _gather(res.results), res.exec_time_ns, res
